# revision 45
# baseline (speedup 1.0000x reference)
"""Multi-head causal attention (B=4, S=2048, D=1024, H=16) on 8 TRN2 cores.

Sharding: core c -> batch c//2, head-group c%2 (8 heads, 512 of the 1024
QKV columns / Wo rows).  Each core runs a fused QKV->attention->out-proj
kernel on its shard; the host sums the two head-group partials per batch.

Per-core layout choices:
  - x is fed pre-transposed (xT [D, S]) so Q^T/K^T come out of the PE in
    [m, s] layout and V in natural [s, m] layout with no on-chip transposes.
    All x chunks and weight slices are prefetched at program start as
    ~128-256KB DMAs spread across the 16 queues.
  - scores are computed transposed (S^T [k, q]); the two heads of a pair
    run as one PE dual-quadrant pair (tile_position rows 0-63 / 64-127),
    softmax exp on ScalarE (scale=1/8 fused, both heads in one op),
    causal mask via gpsimd affine_select on diagonal tiles only.
  - V tiles carry 64 ones-columns per head (cols 64-127, memset once), so
    the attnV matmul replicates the softmax denominator across PSUM
    partitions 64-127.  Normalization is then partition-aligned:
    rec = reciprocal(av[64:128]) (PSUM->SBUF), ct = av[0:64] * rec -- no
    DMA gather, no DRAM bounce, no single-lane staging.
  - all four chunks' out-projections are deferred to the last attention
    chunk, where ScalarE (exp) is otherwise the bottleneck and the PE
    needs independent filler work; fillers are popped between the scores
    pair and the attnV pair so exp latency never stalls the in-order PE.
  - out-proj emits out^T [n, s] in bf16; the host transposes back.
All matmul inputs are bf16 (1 cycle/row on the PE); accumulation stays
fp32 in PSUM.
"""

import numpy as np

B, S, D = 4, 2048, 1024
H, DH = 16, 64
HPC = 8            # heads per core
M = HPC * DH       # 512: per-core qkv out dim / wo in dim
NCORE = 8
CH = 512           # q/s chunk size
NCH = S // CH      # 4
ND = D // 128      # 8  d-tiles (contraction for qkv proj)
NMT = M // 128     # 4  m-tiles (= head pairs)
NKT = S // 128     # 16 k-tiles
NNT = D // 128     # 8  n-tiles (out proj)

LAST_RESULT = None  # BassKernelResults of the most recent run (for test.py)


def _emit(nc, tc, tile, mybir, aps):
    import concourse.bass as bass  # noqa: F401

    f32 = mybir.dt.float32
    bf16 = mybir.dt.bfloat16
    EXP = mybir.ActivationFunctionType.Exp
    xT, wq, wk, wv, wo, outT = aps

    with (
        tc.tile_pool(name="w", bufs=1) as pw,
        tc.tile_pool(name="kv", bufs=1) as pkv,
        tc.tile_pool(name="qt", bufs=2) as pq,
        tc.tile_pool(name="ct", bufs=4) as pct,
        tc.tile_pool(name="x", bufs=1) as px,
        tc.tile_pool(name="u", bufs=6) as pu,
        tc.tile_pool(name="rc", bufs=4) as prc,
        tc.tile_pool(name="o", bufs=2) as po,
        tc.tile_pool(name="ps_mm", bufs=2, space="PSUM") as pp_mm,
        tc.tile_pool(name="ps_sc", bufs=2, space="PSUM") as pp_sc,
        tc.tile_pool(name="ps_av", bufs=2, space="PSUM") as pp_av,
    ):
        # ---- prefetch everything: x chunks + weight d-slices, small DMAs
        # spread across queues, critical-path first ----
        xa = [
            px.tile([128, ND * CH], bf16, name=f"xa{j}", tag=f"xa{j}")
            for j in range(NCH)
        ]
        wq_all = pw.tile([128, ND * M], bf16, name="wqa", tag="wqa")
        wk_all = pw.tile([128, ND * M], bf16, name="wka", tag="wka")
        wv_all = pw.tile([128, ND * M], bf16, name="wva", tag="wva")
        wo_all = pw.tile([128, NMT * D], bf16, name="woa", tag="woa")

        # x0 + wq first (first q-proj), then wk/wv, then x1-3, wo last
        for d in range(ND):
            for e in range(2):
                nc.sync.dma_start(
                    out=xa[0][:, CH * d + 256 * e:CH * d + 256 * (e + 1)],
                    in_=xT[:, CH * d + 256 * e:CH * d + 256 * (e + 1)],
                )
            nc.sync.dma_start(
                out=wq_all[:, M * d:M * (d + 1)], in_=wq[:, M * d:M * (d + 1)]
            )
        for d in range(ND):
            nc.sync.dma_start(
                out=wk_all[:, M * d:M * (d + 1)], in_=wk[:, M * d:M * (d + 1)]
            )
            nc.sync.dma_start(
                out=wv_all[:, M * d:M * (d + 1)], in_=wv[:, M * d:M * (d + 1)]
            )
        for j in range(1, NCH):
            base = ND * CH * j
            for p in range(4):
                nc.sync.dma_start(
                    out=xa[j][:, 2 * CH * p:2 * CH * (p + 1)],
                    in_=xT[:, base + 2 * CH * p:base + 2 * CH * (p + 1)],
                )
        for d in range(NMT):
            nc.sync.dma_start(
                out=wo_all[:, D * d:D * (d + 1)], in_=wo[:, D * d:D * (d + 1)]
            )

        # ---- V storage: [s, 8 heads x (64 V + 64 ones)] ----
        # ones cols 64-127 make the attnV matmul replicate the softmax
        # denominator across PSUM partitions 64-127.
        vau = []
        for st in range(NKT):
            v = pkv.tile([128, HPC * 128], bf16, name=f"vau{st}", tag=f"vau{st}")
            nc.gpsimd.memset(
                v.rearrange("p (h c) -> p h c", c=128)[:, :, 64:128], 1.0
            )
            vau.append(v)

        # warm the PE p-state during the DMA dead window: dummy matmuls on
        # vau[0]'s just-memset ones-columns (gpsimd starts earlier than a
        # DVE memset would), discarded via unread PSUM tiles
        warm = vau[0].rearrange("p (h c) -> p h c", c=128)[:, :, 64:128]
        for w in range(2):
            ps = pp_mm.tile([128, CH], f32, name="pswarm", tag="mm")
            for r in range(4):
                nc.tensor.matmul(
                    ps[0:64, :], lhsT=vau[0][:, 64:128], rhs=warm,
                    start=(r == 0), stop=(r == 3),
                )
        # preload ScalarE's exp table off the critical path
        warm_u = pu.tile([128, 2 * CH], bf16, name="u", tag="u")
        nc.scalar.activation(
            out=warm_u[:, 0:16], in_=vau[0][:, 64:80], func=EXP, scale=0.125
        )

        xt_all = [
            [xa[j][:, CH * d:CH * (d + 1)] for d in range(ND)]
            for j in range(NCH)
        ]
        wq_sb = [wq_all[:, M * d:M * (d + 1)] for d in range(ND)]
        wk_sb = [wk_all[:, M * d:M * (d + 1)] for d in range(ND)]
        wv_sb = [wv_all[:, M * d:M * (d + 1)] for d in range(ND)]
        wo_sb = [wo_all[:, D * t:D * (t + 1)] for t in range(NMT)]

        kt_sb = [[None] * NCH for _ in range(NMT)]
        qt_all = {}   # j -> [4 tiles]
        ct_all = {}   # j -> [4 tiles]

        # ---------- emission units ----------

        def proj_half(ps, w_sb, t, xt, half, kind):
            """4 of the 8 contraction steps of one projection m-tile."""
            for d in range(4 * half, 4 * half + 4):
                if kind == "v":
                    lhsT = xt[d][:, 128 * t:128 * (t + 1)]
                    rhs = w_sb[d]
                else:
                    lhsT = w_sb[d][:, 128 * t:128 * (t + 1)]
                    rhs = xt[d]
                nc.tensor.matmul(
                    ps, lhsT=lhsT, rhs=rhs,
                    start=(d == 0), stop=(d == ND - 1),
                )

        def qkv_unit_lists(j):
            """Unit closures for chunk j's QKV projection, grouped so the
            caller controls interleave order: (q[t], k[t], v[st]) lists."""
            xt = xt_all[j]
            qts = [None] * NMT
            qt_all[j] = qts

            def q_pair(t):
                ps_box = []

                def qa():
                    ps = pp_mm.tile([128, CH], f32, name="psq", tag="mm")
                    ps_box.append(ps)
                    proj_half(ps, wq_sb, t, xt, 0, "q")
                def qb():
                    ps = ps_box[0]
                    proj_half(ps, wq_sb, t, xt, 1, "q")
                    q_t = pq.tile([128, CH], bf16, name=f"q{t}", tag=f"q{t}")
                    nc.vector.tensor_copy(out=q_t, in_=ps)
                    qts[t] = q_t
                return [qa, qb]

            def k_pair(t):
                ps_box = []

                def ka():
                    ps = pp_mm.tile([128, CH], f32, name="psk", tag="mm")
                    ps_box.append(ps)
                    proj_half(ps, wk_sb, t, xt, 0, "k")
                def kb():
                    ps = ps_box[0]
                    proj_half(ps, wk_sb, t, xt, 1, "k")
                    k_t = pkv.tile(
                        [128, CH], bf16, name=f"k{t}_{j}", tag=f"k{t}_{j}"
                    )
                    nc.vector.tensor_copy(out=k_t, in_=ps)
                    kt_sb[t][j] = k_t
                return [ka, kb]

            def v_pair(st):
                ps_box = []

                def va():
                    ps = pp_mm.tile([128, M], f32, name="psv", tag="mm")
                    ps_box.append(ps)
                    proj_half(ps, wv_sb, st, xt, 0, "v")
                def vb():
                    ps = ps_box[0]
                    proj_half(ps, wv_sb, st, xt, 1, "v")
                    g = vau[4 * j + st]
                    nc.vector.tensor_copy(
                        out=g.rearrange("p (h c) -> p h c", c=128)[:, :, 0:64],
                        in_=ps.rearrange("p (h c) -> p h c", c=64),
                    )
                return [va, vb]

            return (
                [q_pair(t) for t in range(NMT)],
                [k_pair(t) for t in range(NMT)],
                [v_pair(st) for st in range(NMT)],
            )

        def qkv_units(j):
            """Chunk j's QKV units in steady-state order: q's, k's, v's."""
            qs, ks, vs = qkv_unit_lists(j)
            units = []
            for t in range(NMT):
                units.extend(qs[t])
            for t in range(NMT):
                units.extend(ks[t])
            for st in range(NMT):
                units.extend(vs[st])
            return units

        def outproj_units(j, tail=False):
            """Emission closures for chunk j's out-projection.  The tail
            variant (final chunk, after the last exp) drains via the
            now-idle ScalarE and splits output DMAs finer so neither DVE
            queueing nor single-queue DMA latency paces the last units."""
            for nt in range(NNT):
                def og(nt=nt, j=j):
                    ct = ct_all[j]
                    ps = pp_mm.tile([128, CH], f32, name="pso", tag="mm")
                    for t in range(NMT):
                        nc.tensor.matmul(
                            ps,
                            lhsT=wo_sb[t][:, 128 * nt:128 * (nt + 1)],
                            rhs=ct[t],
                            start=(t == 0),
                            stop=(t == NMT - 1),
                        )
                    o_sb = po.tile([128, CH], bf16, name="osb", tag="o")
                    nc.vector.tensor_copy(out=o_sb, in_=ps)
                    nsp = 2
                    w = CH // nsp
                    for e in range(nsp):
                        nc.sync.dma_start(
                            out=outT[
                                128 * nt:128 * (nt + 1),
                                CH * j + w * e:CH * j + w * (e + 1),
                            ],
                            in_=o_sb[:, w * e:w * (e + 1)],
                        )
                yield og

        # ---------- chunk 0 QKV up front ----------
        # pad the stream with DMA-independent warm matmuls (into the
        # not-yet-used scores PSUM pool) so input-DMA jitter neither
        # stalls the in-order PE nor resets its p-state ramp
        for i, unit in enumerate(qkv_units(0)):
            unit()
            if i < 12:
                ps = pp_sc.tile([128, 2 * CH], f32, name="sc", tag="sc")
                nc.tensor.matmul(
                    ps[0:64, 0:CH], lhsT=vau[0][:, 64:128], rhs=warm,
                    start=True, stop=True,
                )

        # ---------- main loop: attention(j) with interleaved fillers ----------
        for j in range(NCH):
            fillers = []
            if j + 1 < NCH:
                fillers.extend(qkv_units(j + 1))
            if j == NCH - 1:
                for jo in range(NCH - 1):
                    fillers.extend(outproj_units(jo))
            nkt = 4 * (j + 1)
            # at j=3, under-pop so a few og fillers bridge the final
            # normalize chain before the last out-projection
            n_units = NMT * (nkt + 2) + (4 if j == NCH - 1 else 0)
            n_fill = len(fillers)
            popped = 0
            ucount = 0

            qt = qt_all[j]
            ct = []
            ct_all[j] = ct
            for t in range(NMT):
                av = [
                    pp_av.tile([128, CH], f32, name=f"av{h}", tag="av")
                    for h in range(2)
                ]
                us = {}
                for kt in range(nkt + 2):
                    if kt < nkt:
                        dd = kt - 4 * j      # diagonal index (>=0 on diag)
                        qoff = 128 * dd if dd >= 0 else 0
                        n = CH - qoff
                        ck, ks = kt // 4, (kt % 4) * 128
                        # both heads' scores in one 2-bank PSUM tile
                        sc = pp_sc.tile([128, 2 * CH], f32, name="sc", tag="sc")
                        for h in range(2):
                            pb = 64 * h
                            nc.tensor.matmul(
                                sc[:, CH * h:CH * h + n],
                                lhsT=kt_sb[t][ck][pb:pb + 64, ks:ks + 128],
                                rhs=qt[t][pb:pb + 64, qoff:CH],
                                start=True,
                                stop=True,
                                tile_position=(pb, 0),
                            )
                        u = pu.tile([128, 2 * CH], bf16, name="u", tag="u")
                        scv = sc.rearrange("p (h q) -> p h q", h=2)[:, :, 0:n]
                        uv = u.rearrange("p (h q) -> p h q", h=2)[:, :, 0:n]
                        nc.scalar.activation(out=uv, in_=scv, func=EXP, scale=0.125)
                        if dd >= 0:
                            # keep where q_rel >= k_partition (same mask, both)
                            nc.gpsimd.affine_select(
                                out=uv,
                                in_=uv,
                                compare_op=mybir.AluOpType.is_ge,
                                fill=0.0,
                                base=0,
                                channel_multiplier=-1,
                                pattern=[[0, 2], [1, n]],
                            )
                        us[kt] = (u, qoff, n)
                    # fillers split around the attnV pair: the in-order PE
                    # chews on independent work while ScalarE finishes exp,
                    # and single-buffered mm-PSUM drains overlap attention
                    ucount += 1
                    if fillers and popped < ucount * n_fill // n_units:
                        fillers.pop(0)()
                        popped += 1
                    if kt >= 2:
                        # attnV runs TWO k-tiles behind scores so exp (which
                        # takes ~1.5 score-pairs of ScalarE time) never
                        # stalls the in-order PE stream
                        pkt = kt - 2
                        u_p, qoff_p, n_p = us.pop(pkt)
                        for h in range(2):
                            ha = 2 * t + h
                            nc.tensor.matmul(
                                av[h][:, qoff_p:CH],
                                lhsT=vau[pkt][:, 128 * ha:128 * ha + 128],
                                rhs=u_p[:, CH * h:CH * h + n_p],
                                start=(pkt == 0),
                                stop=(pkt == nkt - 1),
                            )
                    while fillers and popped < ucount * n_fill // n_units:
                        fillers.pop(0)()
                        popped += 1

                # normalize: denominator is replicated on PSUM partitions
                # 64-127 by the ones-columns; partition-aligned recip + mul
                c_t = pct.tile([128, CH], bf16, name=f"c{t}_{j}", tag=f"c{t}")
                for h in range(2):
                    # approx-recip's bitwise seed needs an SBUF fp32 input;
                    # stage the replicated denominator rows out of PSUM first.
                    # ct isn't consumed until the last chunk, so for j<3 the
                    # stage can ride on ScalarE's slack instead of DVE.
                    den = prc.tile([64, CH], f32, name=f"den{h}", tag="den")
                    nc.vector.tensor_copy(out=den, in_=av[h][64:128, :])
                    rec = prc.tile([64, CH], f32, name=f"rec{h}", tag="rec")
                    nc.vector.reciprocal_approx_fast(out=rec, in_=den)
                    nc.vector.tensor_mul(
                        c_t[64 * h:64 * (h + 1), :], av[h][0:64, :], rec
                    )
                ct.append(c_t)

            # leftover fillers for this round
            for f in fillers:
                f()

        # final chunk's out-projection
        for unit in outproj_units(NCH - 1):
            unit()


_PROG = None


def _build():
    global _PROG
    if _PROG is not None:
        return _PROG
    import concourse.bacc as bacc
    import concourse.mybir as mybir
    import concourse.tile as tile

    bf16 = mybir.dt.bfloat16
    nc = bacc.Bacc(
        "TRN2", target_bir_lowering=False, debug=False, enable_asserts=False
    )
    xT = nc.dram_tensor("xT", [128, NCH * ND * CH], bf16, kind="ExternalInput").ap()
    wq = nc.dram_tensor("wq", [128, ND * M], bf16, kind="ExternalInput").ap()
    wk = nc.dram_tensor("wk", [128, ND * M], bf16, kind="ExternalInput").ap()
    wv = nc.dram_tensor("wv", [128, ND * M], bf16, kind="ExternalInput").ap()
    wo = nc.dram_tensor("wo", [128, NMT * D], bf16, kind="ExternalInput").ap()
    outT = nc.dram_tensor("outT", [D, S], bf16, kind="ExternalOutput").ap()

    with tile.TileContext(nc) as tc:
        _emit(nc, tc, tile, mybir, (xT, wq, wk, wv, wo, outT))
    nc.compile()
    _PROG = nc
    return nc


def kernel(x, Wq, Wk, Wv, Wo, bo):
    global LAST_RESULT
    import os

    from concourse.bass_utils import run_bass_kernel_spmd

    x = np.asarray(x, dtype=np.float32)
    Wq = np.asarray(Wq, dtype=np.float32)
    Wk = np.asarray(Wk, dtype=np.float32)
    Wv = np.asarray(Wv, dtype=np.float32)
    Wo = np.asarray(Wo, dtype=np.float32)
    bo = np.asarray(bo, dtype=np.float32)

    nc = _build()

    import ml_dtypes

    bf = ml_dtypes.bfloat16

    def fold_w(w):
        # [(nd p), c] -> [p, (nd c)]
        ndt = w.shape[0] // 128
        return np.ascontiguousarray(
            w.reshape(ndt, 128, w.shape[1]).transpose(1, 0, 2).reshape(128, -1)
        ).astype(bf)

    in_maps = []
    for c in range(NCORE):
        b, g = c // 2, c % 2
        cols = slice(M * g, M * (g + 1))
        xt = x[b].T  # [D, S]
        # [p, (j d s)]: xf[p, j*ND*CH + d*CH + s] = xT[128d+p, CH*j+s]
        xf = (
            xt.reshape(ND, 128, NCH, CH)
            .transpose(1, 2, 0, 3)
            .reshape(128, NCH * ND * CH)
        )
        in_maps.append(
            {
                "xT": np.ascontiguousarray(xf).astype(bf),
                "wq": fold_w(Wq[:, cols]),
                "wk": fold_w(Wk[:, cols]),
                "wv": fold_w(Wv[:, cols]),
                "wo": fold_w(Wo[cols, :]),
            }
        )

    res = run_bass_kernel_spmd(
        nc,
        in_maps,
        list(range(NCORE)),
        trace=bool(os.environ.get("KERNEL_TRACE")),
        tmpdir=os.environ.get("KERNEL_TRACE_DIR") or None,
    )
    LAST_RESULT = res

    out = np.empty((B, S, D), dtype=np.float32)
    for b in range(B):
        acc = res.results[2 * b]["outT"].astype(np.float32) + res.results[
            2 * b + 1
        ]["outT"].astype(np.float32)
        out[b] = acc.T + bo[None, :]
    return out


# revision 46
# speedup vs baseline: 1.0078x; 1.0078x over previous
"""Multi-head causal attention (B=4, S=2048, D=1024, H=16) on 8 TRN2 cores.

Sharding: core c -> batch c//2, head-group c%2 (8 heads, 512 of the 1024
QKV columns / Wo rows).  Each core runs a fused QKV->attention->out-proj
kernel on its shard; the host sums the two head-group partials per batch.

Per-core layout choices:
  - x is fed pre-transposed (xT [D, S]) so Q^T/K^T come out of the PE in
    [m, s] layout and V in natural [s, m] layout with no on-chip transposes.
    All x chunks and weight slices are prefetched at program start as
    ~128-256KB DMAs spread across the 16 queues.
  - scores are computed transposed (S^T [k, q]); the two heads of a pair
    run as one PE dual-quadrant pair (tile_position rows 0-63 / 64-127),
    softmax exp on ScalarE (scale=1/8 fused, both heads in one op),
    causal mask via gpsimd affine_select on diagonal tiles only.
  - V tiles carry 64 ones-columns per head (cols 64-127, memset once), so
    the attnV matmul replicates the softmax denominator across PSUM
    partitions 64-127.  Normalization is then partition-aligned:
    rec = reciprocal(av[64:128]) (PSUM->SBUF), ct = av[0:64] * rec -- no
    DMA gather, no DRAM bounce, no single-lane staging.
  - all four chunks' out-projections are deferred to the last attention
    chunk, where ScalarE (exp) is otherwise the bottleneck and the PE
    needs independent filler work; fillers are popped between the scores
    pair and the attnV pair so exp latency never stalls the in-order PE.
  - out-proj emits out^T [n, s] in bf16; the host transposes back.
All matmul inputs are bf16 (1 cycle/row on the PE); accumulation stays
fp32 in PSUM.
"""

import numpy as np

B, S, D = 4, 2048, 1024
H, DH = 16, 64
HPC = 8            # heads per core
M = HPC * DH       # 512: per-core qkv out dim / wo in dim
NCORE = 8
CH = 512           # q/s chunk size
NCH = S // CH      # 4
ND = D // 128      # 8  d-tiles (contraction for qkv proj)
NMT = M // 128     # 4  m-tiles (= head pairs)
NKT = S // 128     # 16 k-tiles
NNT = D // 128     # 8  n-tiles (out proj)

LAST_RESULT = None  # BassKernelResults of the most recent run (for test.py)


def _emit(nc, tc, tile, mybir, aps):
    import concourse.bass as bass  # noqa: F401

    f32 = mybir.dt.float32
    bf16 = mybir.dt.bfloat16
    EXP = mybir.ActivationFunctionType.Exp
    xT, wq, wk, wv, wo, outT = aps

    with (
        tc.tile_pool(name="w", bufs=1) as pw,
        tc.tile_pool(name="kv", bufs=1) as pkv,
        tc.tile_pool(name="qt", bufs=2) as pq,
        tc.tile_pool(name="ct", bufs=4) as pct,
        tc.tile_pool(name="x", bufs=1) as px,
        tc.tile_pool(name="u", bufs=6) as pu,
        tc.tile_pool(name="rc", bufs=4) as prc,
        tc.tile_pool(name="o", bufs=2) as po,
        tc.tile_pool(name="ps_mm", bufs=2, space="PSUM") as pp_mm,
        tc.tile_pool(name="ps_sc", bufs=2, space="PSUM") as pp_sc,
        tc.tile_pool(name="ps_av", bufs=2, space="PSUM") as pp_av,
    ):
        # ---- prefetch everything: x chunks + weight d-slices, small DMAs
        # spread across queues, critical-path first ----
        xa = [
            px.tile([128, ND * CH], bf16, name=f"xa{j}", tag=f"xa{j}")
            for j in range(NCH)
        ]
        wq_all = pw.tile([128, ND * M], bf16, name="wqa", tag="wqa")
        wk_all = pw.tile([128, ND * M], bf16, name="wka", tag="wka")
        wv_all = pw.tile([128, ND * M], bf16, name="wva", tag="wva")
        wo_all = pw.tile([128, NMT * D], bf16, name="woa", tag="woa")

        # x0 + wq first (first q-proj), then wk/wv, then x1-3, wo last
        for d in range(ND):
            for e in range(2):
                nc.sync.dma_start(
                    out=xa[0][:, CH * d + 256 * e:CH * d + 256 * (e + 1)],
                    in_=xT[:, CH * d + 256 * e:CH * d + 256 * (e + 1)],
                )
            nc.sync.dma_start(
                out=wq_all[:, M * d:M * (d + 1)], in_=wq[:, M * d:M * (d + 1)]
            )
        for d in range(ND):
            nc.sync.dma_start(
                out=wk_all[:, M * d:M * (d + 1)], in_=wk[:, M * d:M * (d + 1)]
            )
            nc.sync.dma_start(
                out=wv_all[:, M * d:M * (d + 1)], in_=wv[:, M * d:M * (d + 1)]
            )
        for j in range(1, NCH):
            base = ND * CH * j
            for p in range(4):
                nc.sync.dma_start(
                    out=xa[j][:, 2 * CH * p:2 * CH * (p + 1)],
                    in_=xT[:, base + 2 * CH * p:base + 2 * CH * (p + 1)],
                )
        for d in range(NMT):
            nc.sync.dma_start(
                out=wo_all[:, D * d:D * (d + 1)], in_=wo[:, D * d:D * (d + 1)]
            )

        # warm the PE p-state during the DMA dead window: dummy matmuls on a
        # memset scratch tile, discarded via unread PSUM tiles
        warm = pw.tile([128, CH], bf16, name="warm", tag="warm")
        nc.vector.memset(warm, 0.0)
        for w in range(2):
            ps = pp_mm.tile([128, CH], f32, name="pswarm", tag="mm")
            for r in range(4):
                nc.tensor.matmul(
                    ps, lhsT=warm[:, 0:128], rhs=warm,
                    start=(r == 0), stop=(r == 3),
                )
        # preload ScalarE's exp table off the critical path
        warm_u = pu.tile([128, 2 * CH], bf16, name="u", tag="u")
        nc.scalar.activation(
            out=warm_u[:, 0:16], in_=warm[:, 0:16], func=EXP, scale=0.125
        )

        xt_all = [
            [xa[j][:, CH * d:CH * (d + 1)] for d in range(ND)]
            for j in range(NCH)
        ]
        wq_sb = [wq_all[:, M * d:M * (d + 1)] for d in range(ND)]
        wk_sb = [wk_all[:, M * d:M * (d + 1)] for d in range(ND)]
        wv_sb = [wv_all[:, M * d:M * (d + 1)] for d in range(ND)]
        wo_sb = [wo_all[:, D * t:D * (t + 1)] for t in range(NMT)]

        # ---- V storage: [s, 8 heads x (64 V + 64 ones)] ----
        # ones cols 64-127 make the attnV matmul replicate the softmax
        # denominator across PSUM partitions 64-127.
        vau = []
        for st in range(NKT):
            v = pkv.tile([128, HPC * 128], bf16, name=f"vau{st}", tag=f"vau{st}")
            nc.gpsimd.memset(
                v.rearrange("p (h c) -> p h c", c=128)[:, :, 64:128], 1.0
            )
            vau.append(v)
        kt_sb = [[None] * NCH for _ in range(NMT)]
        qt_all = {}   # j -> [4 tiles]
        ct_all = {}   # j -> [4 tiles]

        # ---------- emission units ----------

        def proj_half(ps, w_sb, t, xt, half, kind):
            """4 of the 8 contraction steps of one projection m-tile."""
            for d in range(4 * half, 4 * half + 4):
                if kind == "v":
                    lhsT = xt[d][:, 128 * t:128 * (t + 1)]
                    rhs = w_sb[d]
                else:
                    lhsT = w_sb[d][:, 128 * t:128 * (t + 1)]
                    rhs = xt[d]
                nc.tensor.matmul(
                    ps, lhsT=lhsT, rhs=rhs,
                    start=(d == 0), stop=(d == ND - 1),
                )

        def qkv_unit_lists(j):
            """Unit closures for chunk j's QKV projection, grouped so the
            caller controls interleave order: (q[t], k[t], v[st]) lists."""
            xt = xt_all[j]
            qts = [None] * NMT
            qt_all[j] = qts

            def q_pair(t):
                ps_box = []

                def qa():
                    ps = pp_mm.tile([128, CH], f32, name="psq", tag="mm")
                    ps_box.append(ps)
                    proj_half(ps, wq_sb, t, xt, 0, "q")
                def qb():
                    ps = ps_box[0]
                    proj_half(ps, wq_sb, t, xt, 1, "q")
                    q_t = pq.tile([128, CH], bf16, name=f"q{t}", tag=f"q{t}")
                    nc.vector.tensor_copy(out=q_t, in_=ps)
                    qts[t] = q_t
                return [qa, qb]

            def k_pair(t):
                ps_box = []

                def ka():
                    ps = pp_mm.tile([128, CH], f32, name="psk", tag="mm")
                    ps_box.append(ps)
                    proj_half(ps, wk_sb, t, xt, 0, "k")
                def kb():
                    ps = ps_box[0]
                    proj_half(ps, wk_sb, t, xt, 1, "k")
                    k_t = pkv.tile(
                        [128, CH], bf16, name=f"k{t}_{j}", tag=f"k{t}_{j}"
                    )
                    nc.vector.tensor_copy(out=k_t, in_=ps)
                    kt_sb[t][j] = k_t
                return [ka, kb]

            def v_pair(st):
                ps_box = []

                def va():
                    ps = pp_mm.tile([128, M], f32, name="psv", tag="mm")
                    ps_box.append(ps)
                    proj_half(ps, wv_sb, st, xt, 0, "v")
                def vb():
                    ps = ps_box[0]
                    proj_half(ps, wv_sb, st, xt, 1, "v")
                    g = vau[4 * j + st]
                    nc.vector.tensor_copy(
                        out=g.rearrange("p (h c) -> p h c", c=128)[:, :, 0:64],
                        in_=ps.rearrange("p (h c) -> p h c", c=64),
                    )
                return [va, vb]

            return (
                [q_pair(t) for t in range(NMT)],
                [k_pair(t) for t in range(NMT)],
                [v_pair(st) for st in range(NMT)],
            )

        def qkv_units(j):
            """Chunk j's QKV units in steady-state order: q's, k's, v's."""
            qs, ks, vs = qkv_unit_lists(j)
            units = []
            for t in range(NMT):
                units.extend(qs[t])
            for t in range(NMT):
                units.extend(ks[t])
            for st in range(NMT):
                units.extend(vs[st])
            return units

        def outproj_units(j, tail=False):
            """Emission closures for chunk j's out-projection.  The tail
            variant (final chunk, after the last exp) drains via the
            now-idle ScalarE and splits output DMAs finer so neither DVE
            queueing nor single-queue DMA latency paces the last units."""
            for nt in range(NNT):
                def og(nt=nt, j=j):
                    ct = ct_all[j]
                    ps = pp_mm.tile([128, CH], f32, name="pso", tag="mm")
                    for t in range(NMT):
                        nc.tensor.matmul(
                            ps,
                            lhsT=wo_sb[t][:, 128 * nt:128 * (nt + 1)],
                            rhs=ct[t],
                            start=(t == 0),
                            stop=(t == NMT - 1),
                        )
                    o_sb = po.tile([128, CH], bf16, name="osb", tag="o")
                    nc.vector.tensor_copy(out=o_sb, in_=ps)
                    nsp = 2
                    w = CH // nsp
                    for e in range(nsp):
                        nc.sync.dma_start(
                            out=outT[
                                128 * nt:128 * (nt + 1),
                                CH * j + w * e:CH * j + w * (e + 1),
                            ],
                            in_=o_sb[:, w * e:w * (e + 1)],
                        )
                yield og

        # ---------- chunk 0 QKV up front ----------
        # pad the stream with DMA-independent warm matmuls (into the
        # not-yet-used scores PSUM pool) so input-DMA jitter neither
        # stalls the in-order PE nor resets its p-state ramp
        for i, unit in enumerate(qkv_units(0)):
            unit()
            if i < 12:
                ps = pp_sc.tile([128, 2 * CH], f32, name="sc", tag="sc")
                nc.tensor.matmul(
                    ps[:, 0:CH], lhsT=warm[:, 0:128], rhs=warm,
                    start=True, stop=True,
                )

        # ---------- main loop: attention(j) with interleaved fillers ----------
        for j in range(NCH):
            fillers = []
            if j + 1 < NCH:
                fillers.extend(qkv_units(j + 1))
            if j == NCH - 1:
                for jo in range(NCH - 1):
                    fillers.extend(outproj_units(jo))
            nkt = 4 * (j + 1)
            # at j=3, under-pop so a few og fillers bridge the final
            # normalize chain before the last out-projection
            n_units = NMT * (nkt + 2) + (4 if j == NCH - 1 else 0)
            n_fill = len(fillers)
            popped = 0
            ucount = 0

            qt = qt_all[j]
            ct = []
            ct_all[j] = ct
            for t in range(NMT):
                av = [
                    pp_av.tile([128, CH], f32, name=f"av{h}", tag="av")
                    for h in range(2)
                ]
                us = {}
                for kt in range(nkt + 2):
                    if kt < nkt:
                        dd = kt - 4 * j      # diagonal index (>=0 on diag)
                        qoff = 128 * dd if dd >= 0 else 0
                        n = CH - qoff
                        ck, ks = kt // 4, (kt % 4) * 128
                        # both heads' scores in one 2-bank PSUM tile
                        sc = pp_sc.tile([128, 2 * CH], f32, name="sc", tag="sc")
                        for h in range(2):
                            pb = 64 * h
                            nc.tensor.matmul(
                                sc[:, CH * h:CH * h + n],
                                lhsT=kt_sb[t][ck][pb:pb + 64, ks:ks + 128],
                                rhs=qt[t][pb:pb + 64, qoff:CH],
                                start=True,
                                stop=True,
                                tile_position=(pb, 0),
                            )
                        u = pu.tile([128, 2 * CH], bf16, name="u", tag="u")
                        scv = sc.rearrange("p (h q) -> p h q", h=2)[:, :, 0:n]
                        uv = u.rearrange("p (h q) -> p h q", h=2)[:, :, 0:n]
                        nc.scalar.activation(out=uv, in_=scv, func=EXP, scale=0.125)
                        if dd >= 0:
                            # keep where q_rel >= k_partition (same mask, both)
                            nc.gpsimd.affine_select(
                                out=uv,
                                in_=uv,
                                compare_op=mybir.AluOpType.is_ge,
                                fill=0.0,
                                base=0,
                                channel_multiplier=-1,
                                pattern=[[0, 2], [1, n]],
                            )
                        us[kt] = (u, qoff, n)
                    # fillers split around the attnV pair: the in-order PE
                    # chews on independent work while ScalarE finishes exp,
                    # and single-buffered mm-PSUM drains overlap attention
                    ucount += 1
                    if fillers and popped < ucount * n_fill // n_units:
                        fillers.pop(0)()
                        popped += 1
                    if kt >= 2:
                        # attnV runs TWO k-tiles behind scores so exp (which
                        # takes ~1.5 score-pairs of ScalarE time) never
                        # stalls the in-order PE stream
                        pkt = kt - 2
                        u_p, qoff_p, n_p = us.pop(pkt)
                        for h in range(2):
                            ha = 2 * t + h
                            nc.tensor.matmul(
                                av[h][:, qoff_p:CH],
                                lhsT=vau[pkt][:, 128 * ha:128 * ha + 128],
                                rhs=u_p[:, CH * h:CH * h + n_p],
                                start=(pkt == 0),
                                stop=(pkt == nkt - 1),
                            )
                    while fillers and popped < ucount * n_fill // n_units:
                        fillers.pop(0)()
                        popped += 1

                # normalize: denominator is replicated on PSUM partitions
                # 64-127 by the ones-columns; partition-aligned recip + mul
                c_t = pct.tile([128, CH], bf16, name=f"c{t}_{j}", tag=f"c{t}")
                for h in range(2):
                    # approx-recip's bitwise seed needs an SBUF fp32 input;
                    # stage the replicated denominator rows out of PSUM first.
                    # ct isn't consumed until the last chunk, so for j<3 the
                    # stage can ride on ScalarE's slack instead of DVE.
                    den = prc.tile([64, CH], f32, name=f"den{h}", tag="den")
                    if j < 3:
                        nc.scalar.copy(out=den, in_=av[h][64:128, :])
                    else:
                        nc.vector.tensor_copy(out=den, in_=av[h][64:128, :])
                    rec = prc.tile([64, CH], f32, name=f"rec{h}", tag="rec")
                    nc.vector.reciprocal_approx_fast(out=rec, in_=den)
                    nc.vector.tensor_mul(
                        c_t[64 * h:64 * (h + 1), :], av[h][0:64, :], rec
                    )
                ct.append(c_t)

            # leftover fillers for this round
            for f in fillers:
                f()

        # final chunk's out-projection
        for unit in outproj_units(NCH - 1):
            unit()


_PROG = None


def _build():
    global _PROG
    if _PROG is not None:
        return _PROG
    import concourse.bacc as bacc
    import concourse.mybir as mybir
    import concourse.tile as tile

    bf16 = mybir.dt.bfloat16
    nc = bacc.Bacc(
        "TRN2", target_bir_lowering=False, debug=False, enable_asserts=False
    )
    xT = nc.dram_tensor("xT", [128, NCH * ND * CH], bf16, kind="ExternalInput").ap()
    wq = nc.dram_tensor("wq", [128, ND * M], bf16, kind="ExternalInput").ap()
    wk = nc.dram_tensor("wk", [128, ND * M], bf16, kind="ExternalInput").ap()
    wv = nc.dram_tensor("wv", [128, ND * M], bf16, kind="ExternalInput").ap()
    wo = nc.dram_tensor("wo", [128, NMT * D], bf16, kind="ExternalInput").ap()
    outT = nc.dram_tensor("outT", [D, S], bf16, kind="ExternalOutput").ap()

    with tile.TileContext(nc) as tc:
        _emit(nc, tc, tile, mybir, (xT, wq, wk, wv, wo, outT))
    nc.compile()
    _PROG = nc
    return nc


def kernel(x, Wq, Wk, Wv, Wo, bo):
    global LAST_RESULT
    import os

    from concourse.bass_utils import run_bass_kernel_spmd

    x = np.asarray(x, dtype=np.float32)
    Wq = np.asarray(Wq, dtype=np.float32)
    Wk = np.asarray(Wk, dtype=np.float32)
    Wv = np.asarray(Wv, dtype=np.float32)
    Wo = np.asarray(Wo, dtype=np.float32)
    bo = np.asarray(bo, dtype=np.float32)

    nc = _build()

    import ml_dtypes

    bf = ml_dtypes.bfloat16

    def fold_w(w):
        # [(nd p), c] -> [p, (nd c)]
        ndt = w.shape[0] // 128
        return np.ascontiguousarray(
            w.reshape(ndt, 128, w.shape[1]).transpose(1, 0, 2).reshape(128, -1)
        ).astype(bf)

    in_maps = []
    for c in range(NCORE):
        b, g = c // 2, c % 2
        cols = slice(M * g, M * (g + 1))
        xt = x[b].T  # [D, S]
        # [p, (j d s)]: xf[p, j*ND*CH + d*CH + s] = xT[128d+p, CH*j+s]
        xf = (
            xt.reshape(ND, 128, NCH, CH)
            .transpose(1, 2, 0, 3)
            .reshape(128, NCH * ND * CH)
        )
        in_maps.append(
            {
                "xT": np.ascontiguousarray(xf).astype(bf),
                "wq": fold_w(Wq[:, cols]),
                "wk": fold_w(Wk[:, cols]),
                "wv": fold_w(Wv[:, cols]),
                "wo": fold_w(Wo[cols, :]),
            }
        )

    res = run_bass_kernel_spmd(
        nc,
        in_maps,
        list(range(NCORE)),
        trace=bool(os.environ.get("KERNEL_TRACE")),
        tmpdir=os.environ.get("KERNEL_TRACE_DIR") or None,
    )
    LAST_RESULT = res

    out = np.empty((B, S, D), dtype=np.float32)
    for b in range(B):
        acc = res.results[2 * b]["outT"].astype(np.float32) + res.results[
            2 * b + 1
        ]["outT"].astype(np.float32)
        out[b] = acc.T + bo[None, :]
    return out


# revision 47
# speedup vs baseline: 1.0089x; 1.0012x over previous
"""Multi-head causal attention (B=4, S=2048, D=1024, H=16) on 8 TRN2 cores.

Sharding: core c -> batch c//2, head-group c%2 (8 heads, 512 of the 1024
QKV columns / Wo rows).  Each core runs a fused QKV->attention->out-proj
kernel on its shard; the host sums the two head-group partials per batch.

Per-core layout choices:
  - x is fed pre-transposed (xT [D, S]) so Q^T/K^T come out of the PE in
    [m, s] layout and V in natural [s, m] layout with no on-chip transposes.
    All x chunks and weight slices are prefetched at program start as
    ~128-256KB DMAs spread across the 16 queues.
  - scores are computed transposed (S^T [k, q]); the two heads of a pair
    run as one PE dual-quadrant pair (tile_position rows 0-63 / 64-127),
    softmax exp on ScalarE (scale=1/8 fused, both heads in one op),
    causal mask via gpsimd affine_select on diagonal tiles only.
  - V tiles carry 64 ones-columns per head (cols 64-127, memset once), so
    the attnV matmul replicates the softmax denominator across PSUM
    partitions 64-127.  Normalization is then partition-aligned:
    rec = reciprocal(av[64:128]) (PSUM->SBUF), ct = av[0:64] * rec -- no
    DMA gather, no DRAM bounce, no single-lane staging.
  - all four chunks' out-projections are deferred to the last attention
    chunk, where ScalarE (exp) is otherwise the bottleneck and the PE
    needs independent filler work; fillers are popped between the scores
    pair and the attnV pair so exp latency never stalls the in-order PE.
  - out-proj emits out^T [n, s] in bf16; the host transposes back.
All matmul inputs are bf16 (1 cycle/row on the PE); accumulation stays
fp32 in PSUM.
"""

import numpy as np

B, S, D = 4, 2048, 1024
H, DH = 16, 64
HPC = 8            # heads per core
M = HPC * DH       # 512: per-core qkv out dim / wo in dim
NCORE = 8
CH = 512           # q/s chunk size
NCH = S // CH      # 4
ND = D // 128      # 8  d-tiles (contraction for qkv proj)
NMT = M // 128     # 4  m-tiles (= head pairs)
NKT = S // 128     # 16 k-tiles
NNT = D // 128     # 8  n-tiles (out proj)

LAST_RESULT = None  # BassKernelResults of the most recent run (for test.py)


def _emit(nc, tc, tile, mybir, aps):
    import concourse.bass as bass  # noqa: F401

    f32 = mybir.dt.float32
    bf16 = mybir.dt.bfloat16
    EXP = mybir.ActivationFunctionType.Exp
    xT, wq, wk, wv, wo, outT = aps

    with (
        tc.tile_pool(name="w", bufs=1) as pw,
        tc.tile_pool(name="kv", bufs=1) as pkv,
        tc.tile_pool(name="qt", bufs=2) as pq,
        tc.tile_pool(name="ct", bufs=4) as pct,
        tc.tile_pool(name="x", bufs=1) as px,
        tc.tile_pool(name="u", bufs=6) as pu,
        tc.tile_pool(name="rc", bufs=4) as prc,
        tc.tile_pool(name="o", bufs=2) as po,
        tc.tile_pool(name="ps_mm", bufs=2, space="PSUM") as pp_mm,
        tc.tile_pool(name="ps_sc", bufs=2, space="PSUM") as pp_sc,
        tc.tile_pool(name="ps_av", bufs=2, space="PSUM") as pp_av,
    ):
        # ---- prefetch everything: x chunks + weight d-slices, small DMAs
        # spread across queues, critical-path first ----
        xa = [
            px.tile([128, ND * CH], bf16, name=f"xa{j}", tag=f"xa{j}")
            for j in range(NCH)
        ]
        wq_all = pw.tile([128, ND * M], bf16, name="wqa", tag="wqa")
        wk_all = pw.tile([128, ND * M], bf16, name="wka", tag="wka")
        wv_all = pw.tile([128, ND * M], bf16, name="wva", tag="wva")
        wo_all = pw.tile([128, NMT * D], bf16, name="woa", tag="woa")

        # x0 + wq first (first q-proj), then wk/wv, then x1-3, wo last
        for d in range(ND):
            for e in range(2):
                nc.sync.dma_start(
                    out=xa[0][:, CH * d + 256 * e:CH * d + 256 * (e + 1)],
                    in_=xT[:, CH * d + 256 * e:CH * d + 256 * (e + 1)],
                )
            nc.sync.dma_start(
                out=wq_all[:, M * d:M * (d + 1)], in_=wq[:, M * d:M * (d + 1)]
            )
        for d in range(ND):
            nc.sync.dma_start(
                out=wk_all[:, M * d:M * (d + 1)], in_=wk[:, M * d:M * (d + 1)]
            )
            nc.sync.dma_start(
                out=wv_all[:, M * d:M * (d + 1)], in_=wv[:, M * d:M * (d + 1)]
            )
        for j in range(1, NCH):
            base = ND * CH * j
            for p in range(4):
                nc.sync.dma_start(
                    out=xa[j][:, 2 * CH * p:2 * CH * (p + 1)],
                    in_=xT[:, base + 2 * CH * p:base + 2 * CH * (p + 1)],
                )
        for d in range(NMT):
            nc.sync.dma_start(
                out=wo_all[:, D * d:D * (d + 1)], in_=wo[:, D * d:D * (d + 1)]
            )

        # warm the PE p-state during the DMA dead window: dummy matmuls on a
        # memset scratch tile, discarded via unread PSUM tiles
        warm = pw.tile([128, CH], bf16, name="warm", tag="warm")
        nc.vector.memset(warm, 0.0)
        for w in range(2):
            ps = pp_mm.tile([128, CH], f32, name="pswarm", tag="mm")
            for r in range(4):
                nc.tensor.matmul(
                    ps, lhsT=warm[:, 0:128], rhs=warm,
                    start=(r == 0), stop=(r == 3),
                )
        # preload ScalarE's exp table off the critical path
        warm_u = pu.tile([128, 2 * CH], bf16, name="u", tag="u")
        nc.scalar.activation(
            out=warm_u[:, 0:16], in_=warm[:, 0:16], func=EXP, scale=0.125
        )

        xt_all = [
            [xa[j][:, CH * d:CH * (d + 1)] for d in range(ND)]
            for j in range(NCH)
        ]
        wq_sb = [wq_all[:, M * d:M * (d + 1)] for d in range(ND)]
        wk_sb = [wk_all[:, M * d:M * (d + 1)] for d in range(ND)]
        wv_sb = [wv_all[:, M * d:M * (d + 1)] for d in range(ND)]
        wo_sb = [wo_all[:, D * t:D * (t + 1)] for t in range(NMT)]

        # ---- V storage: [s, 8 heads x (64 V + 64 ones)] ----
        # ones cols 64-127 make the attnV matmul replicate the softmax
        # denominator across PSUM partitions 64-127.
        vau = []
        for st in range(NKT):
            v = pkv.tile([128, HPC * 128], bf16, name=f"vau{st}", tag=f"vau{st}")
            nc.gpsimd.memset(
                v.rearrange("p (h c) -> p h c", c=128)[:, :, 64:128], 1.0
            )
            vau.append(v)
        kt_sb = [[None] * NCH for _ in range(NMT)]
        qt_all = {}   # j -> [4 tiles]
        ct_all = {}   # j -> [4 tiles]

        # ---------- emission units ----------

        def proj_half(ps, w_sb, t, xt, half, kind):
            """4 of the 8 contraction steps of one projection m-tile."""
            for d in range(4 * half, 4 * half + 4):
                if kind == "v":
                    lhsT = xt[d][:, 128 * t:128 * (t + 1)]
                    rhs = w_sb[d]
                else:
                    lhsT = w_sb[d][:, 128 * t:128 * (t + 1)]
                    rhs = xt[d]
                nc.tensor.matmul(
                    ps, lhsT=lhsT, rhs=rhs,
                    start=(d == 0), stop=(d == ND - 1),
                )

        def qkv_unit_lists(j):
            """Unit closures for chunk j's QKV projection, grouped so the
            caller controls interleave order: (q[t], k[t], v[st]) lists."""
            xt = xt_all[j]
            qts = [None] * NMT
            qt_all[j] = qts

            def q_pair(t):
                ps_box = []

                def qa():
                    ps = pp_mm.tile([128, CH], f32, name="psq", tag="mm")
                    ps_box.append(ps)
                    proj_half(ps, wq_sb, t, xt, 0, "q")
                def qb():
                    ps = ps_box[0]
                    proj_half(ps, wq_sb, t, xt, 1, "q")
                    q_t = pq.tile([128, CH], bf16, name=f"q{t}", tag=f"q{t}")
                    nc.vector.tensor_copy(out=q_t, in_=ps)
                    qts[t] = q_t
                return [qa, qb]

            def k_pair(t):
                ps_box = []

                def ka():
                    ps = pp_mm.tile([128, CH], f32, name="psk", tag="mm")
                    ps_box.append(ps)
                    proj_half(ps, wk_sb, t, xt, 0, "k")
                def kb():
                    ps = ps_box[0]
                    proj_half(ps, wk_sb, t, xt, 1, "k")
                    k_t = pkv.tile(
                        [128, CH], bf16, name=f"k{t}_{j}", tag=f"k{t}_{j}"
                    )
                    nc.vector.tensor_copy(out=k_t, in_=ps)
                    kt_sb[t][j] = k_t
                return [ka, kb]

            def v_pair(st):
                ps_box = []

                def va():
                    ps = pp_mm.tile([128, M], f32, name="psv", tag="mm")
                    ps_box.append(ps)
                    proj_half(ps, wv_sb, st, xt, 0, "v")
                def vb():
                    ps = ps_box[0]
                    proj_half(ps, wv_sb, st, xt, 1, "v")
                    g = vau[4 * j + st]
                    nc.vector.tensor_copy(
                        out=g.rearrange("p (h c) -> p h c", c=128)[:, :, 0:64],
                        in_=ps.rearrange("p (h c) -> p h c", c=64),
                    )
                return [va, vb]

            return (
                [q_pair(t) for t in range(NMT)],
                [k_pair(t) for t in range(NMT)],
                [v_pair(st) for st in range(NMT)],
            )

        def qkv_units(j):
            """Chunk j's QKV units in steady-state order: q's, k's, v's."""
            qs, ks, vs = qkv_unit_lists(j)
            units = []
            for t in range(NMT):
                units.extend(qs[t])
            for t in range(NMT):
                units.extend(ks[t])
            for st in range(NMT):
                units.extend(vs[st])
            return units

        def outproj_units(j, tail=False):
            """Emission closures for chunk j's out-projection.  The tail
            variant (final chunk, after the last exp) drains via the
            now-idle ScalarE and splits output DMAs finer so neither DVE
            queueing nor single-queue DMA latency paces the last units."""
            for nt in range(NNT):
                def og(nt=nt, j=j):
                    ct = ct_all[j]
                    ps = pp_mm.tile([128, CH], f32, name="pso", tag="mm")
                    for t in range(NMT):
                        nc.tensor.matmul(
                            ps,
                            lhsT=wo_sb[t][:, 128 * nt:128 * (nt + 1)],
                            rhs=ct[t],
                            start=(t == 0),
                            stop=(t == NMT - 1),
                        )
                    o_sb = po.tile([128, CH], bf16, name="osb", tag="o")
                    if tail:
                        nc.scalar.copy(out=o_sb, in_=ps)
                    else:
                        nc.vector.tensor_copy(out=o_sb, in_=ps)
                    nsp = 2
                    w = CH // nsp
                    for e in range(nsp):
                        nc.sync.dma_start(
                            out=outT[
                                128 * nt:128 * (nt + 1),
                                CH * j + w * e:CH * j + w * (e + 1),
                            ],
                            in_=o_sb[:, w * e:w * (e + 1)],
                        )
                yield og

        # ---------- chunk 0 QKV up front ----------
        # pad the stream with DMA-independent warm matmuls (into the
        # not-yet-used scores PSUM pool) so input-DMA jitter neither
        # stalls the in-order PE nor resets its p-state ramp
        for i, unit in enumerate(qkv_units(0)):
            unit()
            if i < 12:
                ps = pp_sc.tile([128, 2 * CH], f32, name="sc", tag="sc")
                nc.tensor.matmul(
                    ps[:, 0:CH], lhsT=warm[:, 0:128], rhs=warm,
                    start=True, stop=True,
                )

        # ---------- main loop: attention(j) with interleaved fillers ----------
        for j in range(NCH):
            fillers = []
            if j + 1 < NCH:
                fillers.extend(qkv_units(j + 1))
            if j == NCH - 1:
                for jo in range(NCH - 1):
                    fillers.extend(outproj_units(jo))
            nkt = 4 * (j + 1)
            # at j=3, under-pop so a few og fillers bridge the final
            # normalize chain before the last out-projection
            n_units = NMT * (nkt + 2) + (4 if j == NCH - 1 else 0)
            n_fill = len(fillers)
            popped = 0
            ucount = 0

            qt = qt_all[j]
            ct = []
            ct_all[j] = ct
            for t in range(NMT):
                av = [
                    pp_av.tile([128, CH], f32, name=f"av{h}", tag="av")
                    for h in range(2)
                ]
                us = {}
                for kt in range(nkt + 2):
                    if kt < nkt:
                        dd = kt - 4 * j      # diagonal index (>=0 on diag)
                        qoff = 128 * dd if dd >= 0 else 0
                        n = CH - qoff
                        ck, ks = kt // 4, (kt % 4) * 128
                        # both heads' scores in one 2-bank PSUM tile
                        sc = pp_sc.tile([128, 2 * CH], f32, name="sc", tag="sc")
                        for h in range(2):
                            pb = 64 * h
                            nc.tensor.matmul(
                                sc[:, CH * h:CH * h + n],
                                lhsT=kt_sb[t][ck][pb:pb + 64, ks:ks + 128],
                                rhs=qt[t][pb:pb + 64, qoff:CH],
                                start=True,
                                stop=True,
                                tile_position=(pb, 0),
                            )
                        u = pu.tile([128, 2 * CH], bf16, name="u", tag="u")
                        scv = sc.rearrange("p (h q) -> p h q", h=2)[:, :, 0:n]
                        uv = u.rearrange("p (h q) -> p h q", h=2)[:, :, 0:n]
                        nc.scalar.activation(out=uv, in_=scv, func=EXP, scale=0.125)
                        if dd >= 0:
                            # keep where q_rel >= k_partition (same mask, both)
                            nc.gpsimd.affine_select(
                                out=uv,
                                in_=uv,
                                compare_op=mybir.AluOpType.is_ge,
                                fill=0.0,
                                base=0,
                                channel_multiplier=-1,
                                pattern=[[0, 2], [1, n]],
                            )
                        us[kt] = (u, qoff, n)
                    # fillers split around the attnV pair: the in-order PE
                    # chews on independent work while ScalarE finishes exp,
                    # and single-buffered mm-PSUM drains overlap attention
                    ucount += 1
                    if fillers and popped < ucount * n_fill // n_units:
                        fillers.pop(0)()
                        popped += 1
                    if kt >= 2:
                        # attnV runs TWO k-tiles behind scores so exp (which
                        # takes ~1.5 score-pairs of ScalarE time) never
                        # stalls the in-order PE stream
                        pkt = kt - 2
                        u_p, qoff_p, n_p = us.pop(pkt)
                        for h in range(2):
                            ha = 2 * t + h
                            nc.tensor.matmul(
                                av[h][:, qoff_p:CH],
                                lhsT=vau[pkt][:, 128 * ha:128 * ha + 128],
                                rhs=u_p[:, CH * h:CH * h + n_p],
                                start=(pkt == 0),
                                stop=(pkt == nkt - 1),
                            )
                    while fillers and popped < ucount * n_fill // n_units:
                        fillers.pop(0)()
                        popped += 1

                # normalize: denominator is replicated on PSUM partitions
                # 64-127 by the ones-columns; partition-aligned recip + mul
                c_t = pct.tile([128, CH], bf16, name=f"c{t}_{j}", tag=f"c{t}")
                for h in range(2):
                    # approx-recip's bitwise seed needs an SBUF fp32 input;
                    # stage the replicated denominator rows out of PSUM first.
                    # ct isn't consumed until the last chunk, so for j<3 the
                    # stage can ride on ScalarE's slack instead of DVE.
                    den = prc.tile([64, CH], f32, name=f"den{h}", tag="den")
                    if j < 3:
                        nc.scalar.copy(out=den, in_=av[h][64:128, :])
                    else:
                        nc.vector.tensor_copy(out=den, in_=av[h][64:128, :])
                    rec = prc.tile([64, CH], f32, name=f"rec{h}", tag="rec")
                    nc.vector.reciprocal_approx_fast(out=rec, in_=den)
                    nc.vector.tensor_mul(
                        c_t[64 * h:64 * (h + 1), :], av[h][0:64, :], rec
                    )
                ct.append(c_t)

            # leftover fillers for this round
            for f in fillers:
                f()

        # final chunk's out-projection
        for unit in outproj_units(NCH - 1, tail=True):
            unit()


_PROG = None


def _build():
    global _PROG
    if _PROG is not None:
        return _PROG
    import concourse.bacc as bacc
    import concourse.mybir as mybir
    import concourse.tile as tile

    bf16 = mybir.dt.bfloat16
    nc = bacc.Bacc(
        "TRN2", target_bir_lowering=False, debug=False, enable_asserts=False
    )
    xT = nc.dram_tensor("xT", [128, NCH * ND * CH], bf16, kind="ExternalInput").ap()
    wq = nc.dram_tensor("wq", [128, ND * M], bf16, kind="ExternalInput").ap()
    wk = nc.dram_tensor("wk", [128, ND * M], bf16, kind="ExternalInput").ap()
    wv = nc.dram_tensor("wv", [128, ND * M], bf16, kind="ExternalInput").ap()
    wo = nc.dram_tensor("wo", [128, NMT * D], bf16, kind="ExternalInput").ap()
    outT = nc.dram_tensor("outT", [D, S], bf16, kind="ExternalOutput").ap()

    with tile.TileContext(nc) as tc:
        _emit(nc, tc, tile, mybir, (xT, wq, wk, wv, wo, outT))
    nc.compile()
    _PROG = nc
    return nc


def kernel(x, Wq, Wk, Wv, Wo, bo):
    global LAST_RESULT
    import os

    from concourse.bass_utils import run_bass_kernel_spmd

    x = np.asarray(x, dtype=np.float32)
    Wq = np.asarray(Wq, dtype=np.float32)
    Wk = np.asarray(Wk, dtype=np.float32)
    Wv = np.asarray(Wv, dtype=np.float32)
    Wo = np.asarray(Wo, dtype=np.float32)
    bo = np.asarray(bo, dtype=np.float32)

    nc = _build()

    import ml_dtypes

    bf = ml_dtypes.bfloat16

    def fold_w(w):
        # [(nd p), c] -> [p, (nd c)]
        ndt = w.shape[0] // 128
        return np.ascontiguousarray(
            w.reshape(ndt, 128, w.shape[1]).transpose(1, 0, 2).reshape(128, -1)
        ).astype(bf)

    in_maps = []
    for c in range(NCORE):
        b, g = c // 2, c % 2
        cols = slice(M * g, M * (g + 1))
        xt = x[b].T  # [D, S]
        # [p, (j d s)]: xf[p, j*ND*CH + d*CH + s] = xT[128d+p, CH*j+s]
        xf = (
            xt.reshape(ND, 128, NCH, CH)
            .transpose(1, 2, 0, 3)
            .reshape(128, NCH * ND * CH)
        )
        in_maps.append(
            {
                "xT": np.ascontiguousarray(xf).astype(bf),
                "wq": fold_w(Wq[:, cols]),
                "wk": fold_w(Wk[:, cols]),
                "wv": fold_w(Wv[:, cols]),
                "wo": fold_w(Wo[cols, :]),
            }
        )

    res = run_bass_kernel_spmd(
        nc,
        in_maps,
        list(range(NCORE)),
        trace=bool(os.environ.get("KERNEL_TRACE")),
        tmpdir=os.environ.get("KERNEL_TRACE_DIR") or None,
    )
    LAST_RESULT = res

    out = np.empty((B, S, D), dtype=np.float32)
    for b in range(B):
        acc = res.results[2 * b]["outT"].astype(np.float32) + res.results[
            2 * b + 1
        ]["outT"].astype(np.float32)
        out[b] = acc.T + bo[None, :]
    return out


# revision 50
# speedup vs baseline: 1.0266x; 1.0175x over previous
"""Multi-head causal attention (B=4, S=2048, D=1024, H=16) on 8 TRN2 cores.

Sharding: core c -> batch c//2, head-group c%2 (8 heads, 512 of the 1024
QKV columns / Wo rows).  Each core runs a fused QKV->attention->out-proj
kernel on its shard; the host sums the two head-group partials per batch.

Per-core layout choices:
  - x is fed pre-transposed (xT [D, S]) so Q^T/K^T come out of the PE in
    [m, s] layout and V in natural [s, m] layout with no on-chip transposes.
    All x chunks and weight slices are prefetched at program start as
    ~128-256KB DMAs spread across the 16 queues.
  - scores are computed transposed (S^T [k, q]); the two heads of a pair
    run as one PE dual-quadrant pair (tile_position rows 0-63 / 64-127),
    softmax exp on ScalarE (scale=1/8 fused, both heads in one op),
    causal mask via gpsimd affine_select on diagonal tiles only.
  - V tiles carry 64 ones-columns per head (cols 64-127, memset once), so
    the attnV matmul replicates the softmax denominator across PSUM
    partitions 64-127.  Normalization is then partition-aligned:
    rec = reciprocal(av[64:128]) (PSUM->SBUF), ct = av[0:64] * rec -- no
    DMA gather, no DRAM bounce, no single-lane staging.
  - all four chunks' out-projections are deferred to the last attention
    chunk, where ScalarE (exp) is otherwise the bottleneck and the PE
    needs independent filler work; fillers are popped between the scores
    pair and the attnV pair so exp latency never stalls the in-order PE.
  - out-proj emits out^T [n, s] in bf16; the host transposes back.
All matmul inputs are bf16 (1 cycle/row on the PE); accumulation stays
fp32 in PSUM.
"""

import numpy as np

B, S, D = 4, 2048, 1024
H, DH = 16, 64
HPC = 8            # heads per core
M = HPC * DH       # 512: per-core qkv out dim / wo in dim
NCORE = 8
CH = 512           # q/s chunk size
NCH = S // CH      # 4
ND = D // 128      # 8  d-tiles (contraction for qkv proj)
NMT = M // 128     # 4  m-tiles (= head pairs)
NKT = S // 128     # 16 k-tiles
NNT = D // 128     # 8  n-tiles (out proj)

LAST_RESULT = None  # BassKernelResults of the most recent run (for test.py)


def _emit(nc, tc, tile, mybir, aps):
    import concourse.bass as bass  # noqa: F401

    f32 = mybir.dt.float32
    bf16 = mybir.dt.bfloat16
    EXP = mybir.ActivationFunctionType.Exp
    xT, wq, wk, wv, wo, outT = aps

    with (
        tc.tile_pool(name="w", bufs=1) as pw,
        tc.tile_pool(name="kv", bufs=1) as pkv,
        tc.tile_pool(name="qt", bufs=2) as pq,
        tc.tile_pool(name="ct", bufs=4) as pct,
        tc.tile_pool(name="x", bufs=1) as px,
        tc.tile_pool(name="u", bufs=6) as pu,
        tc.tile_pool(name="rc", bufs=4) as prc,
        tc.tile_pool(name="o", bufs=6) as po,
        tc.tile_pool(name="ps_mm", bufs=2, space="PSUM") as pp_mm,
        tc.tile_pool(name="ps_sc", bufs=2, space="PSUM") as pp_sc,
        tc.tile_pool(name="ps_av", bufs=2, space="PSUM") as pp_av,
    ):
        # ---- prefetch everything: x chunks + weight d-slices, small DMAs
        # spread across queues, critical-path first ----
        xa = [
            px.tile([128, ND * CH], bf16, name=f"xa{j}", tag=f"xa{j}")
            for j in range(NCH)
        ]
        wq_all = pw.tile([128, ND * M], bf16, name="wqa", tag="wqa")
        wk_all = pw.tile([128, ND * M], bf16, name="wka", tag="wka")
        wv_all = pw.tile([128, ND * M], bf16, name="wva", tag="wva")
        wo_all = pw.tile([128, NMT * D], bf16, name="woa", tag="woa")

        # x0 + wq first (first q-proj), then wk/wv, then x1-3, wo last
        for d in range(ND):
            for e in range(2):
                nc.sync.dma_start(
                    out=xa[0][:, CH * d + 256 * e:CH * d + 256 * (e + 1)],
                    in_=xT[:, CH * d + 256 * e:CH * d + 256 * (e + 1)],
                )
            nc.sync.dma_start(
                out=wq_all[:, M * d:M * (d + 1)], in_=wq[:, M * d:M * (d + 1)]
            )
        for d in range(ND):
            nc.sync.dma_start(
                out=wk_all[:, M * d:M * (d + 1)], in_=wk[:, M * d:M * (d + 1)]
            )
            nc.sync.dma_start(
                out=wv_all[:, M * d:M * (d + 1)], in_=wv[:, M * d:M * (d + 1)]
            )
        for j in range(1, NCH):
            base = ND * CH * j
            for p in range(4):
                nc.sync.dma_start(
                    out=xa[j][:, 2 * CH * p:2 * CH * (p + 1)],
                    in_=xT[:, base + 2 * CH * p:base + 2 * CH * (p + 1)],
                )
        for d in range(NMT):
            nc.sync.dma_start(
                out=wo_all[:, D * d:D * (d + 1)], in_=wo[:, D * d:D * (d + 1)]
            )

        # warm the PE p-state during the DMA dead window: dummy matmuls on a
        # memset scratch tile, discarded via unread PSUM tiles
        warm = pw.tile([128, CH], bf16, name="warm", tag="warm")
        nc.vector.memset(warm, 0.0)
        for w in range(2):
            ps = pp_mm.tile([128, CH], f32, name="pswarm", tag="mm")
            for r in range(4):
                nc.tensor.matmul(
                    ps, lhsT=warm[:, 0:128], rhs=warm,
                    start=(r == 0), stop=(r == 3),
                )
        # preload ScalarE's exp table off the critical path
        warm_u = pu.tile([128, 2 * CH], bf16, name="u", tag="u")
        nc.scalar.activation(
            out=warm_u[:, 0:16], in_=warm[:, 0:16], func=EXP, scale=0.125
        )

        xt_all = [
            [xa[j][:, CH * d:CH * (d + 1)] for d in range(ND)]
            for j in range(NCH)
        ]
        wq_sb = [wq_all[:, M * d:M * (d + 1)] for d in range(ND)]
        wk_sb = [wk_all[:, M * d:M * (d + 1)] for d in range(ND)]
        wv_sb = [wv_all[:, M * d:M * (d + 1)] for d in range(ND)]
        wo_sb = [wo_all[:, D * t:D * (t + 1)] for t in range(NMT)]

        # ---- V storage: [s, 8 heads x (64 V + 64 ones)] ----
        # ones cols 64-127 make the attnV matmul replicate the softmax
        # denominator across PSUM partitions 64-127.
        vau = []
        for st in range(NKT):
            v = pkv.tile([128, HPC * 128], bf16, name=f"vau{st}", tag=f"vau{st}")
            nc.gpsimd.memset(
                v.rearrange("p (h c) -> p h c", c=128)[:, :, 64:128], 1.0
            )
            vau.append(v)
        kt_sb = [[None] * NCH for _ in range(NMT)]
        qt_all = {}   # j -> [4 tiles]
        ct_all = {}   # j -> [4 tiles]

        # ---------- emission units ----------

        def proj_half(ps, w_sb, t, xt, half, kind):
            """4 of the 8 contraction steps of one projection m-tile."""
            for d in range(4 * half, 4 * half + 4):
                if kind == "v":
                    lhsT = xt[d][:, 128 * t:128 * (t + 1)]
                    rhs = w_sb[d]
                else:
                    lhsT = w_sb[d][:, 128 * t:128 * (t + 1)]
                    rhs = xt[d]
                nc.tensor.matmul(
                    ps, lhsT=lhsT, rhs=rhs,
                    start=(d == 0), stop=(d == ND - 1),
                )

        def qkv_unit_lists(j):
            """Unit closures for chunk j's QKV projection, grouped so the
            caller controls interleave order: (q[t], k[t], v[st]) lists."""
            xt = xt_all[j]
            qts = [None] * NMT
            qt_all[j] = qts

            def q_pair(t):
                ps_box = []

                def qa():
                    ps = pp_mm.tile([128, CH], f32, name="psq", tag="mm")
                    ps_box.append(ps)
                    proj_half(ps, wq_sb, t, xt, 0, "q")
                def qb():
                    ps = ps_box[0]
                    proj_half(ps, wq_sb, t, xt, 1, "q")
                    q_t = pq.tile([128, CH], bf16, name=f"q{t}", tag=f"q{t}")
                    nc.vector.tensor_copy(out=q_t, in_=ps)
                    qts[t] = q_t
                return [qa, qb]

            def k_pair(t):
                ps_box = []

                def ka():
                    ps = pp_mm.tile([128, CH], f32, name="psk", tag="mm")
                    ps_box.append(ps)
                    proj_half(ps, wk_sb, t, xt, 0, "k")
                def kb():
                    ps = ps_box[0]
                    proj_half(ps, wk_sb, t, xt, 1, "k")
                    k_t = pkv.tile(
                        [128, CH], bf16, name=f"k{t}_{j}", tag=f"k{t}_{j}"
                    )
                    nc.vector.tensor_copy(out=k_t, in_=ps)
                    kt_sb[t][j] = k_t
                return [ka, kb]

            def v_pair(st):
                ps_box = []

                def va():
                    ps = pp_mm.tile([128, M], f32, name="psv", tag="mm")
                    ps_box.append(ps)
                    proj_half(ps, wv_sb, st, xt, 0, "v")
                def vb():
                    ps = ps_box[0]
                    proj_half(ps, wv_sb, st, xt, 1, "v")
                    g = vau[4 * j + st]
                    nc.vector.tensor_copy(
                        out=g.rearrange("p (h c) -> p h c", c=128)[:, :, 0:64],
                        in_=ps.rearrange("p (h c) -> p h c", c=64),
                    )
                return [va, vb]

            return (
                [q_pair(t) for t in range(NMT)],
                [k_pair(t) for t in range(NMT)],
                [v_pair(st) for st in range(NMT)],
            )

        def qkv_units(j):
            """Chunk j's QKV units in steady-state order: q's, k's, v's."""
            qs, ks, vs = qkv_unit_lists(j)
            units = []
            for t in range(NMT):
                units.extend(qs[t])
            for t in range(NMT):
                units.extend(ks[t])
            for st in range(NMT):
                units.extend(vs[st])
            return units

        def outproj_units(j, tail=False):
            """Emission closures for chunk j's out-projection.  The tail
            variant (final chunk, after the last exp) drains via the
            now-idle ScalarE and splits output DMAs finer so neither DVE
            queueing nor single-queue DMA latency paces the last units."""
            for nt in range(NNT):
                def og(nt=nt, j=j):
                    ct = ct_all[j]
                    ps = pp_mm.tile([128, CH], f32, name="pso", tag="mm")
                    for t in range(NMT):
                        nc.tensor.matmul(
                            ps,
                            lhsT=wo_sb[t][:, 128 * nt:128 * (nt + 1)],
                            rhs=ct[t],
                            start=(t == 0),
                            stop=(t == NMT - 1),
                        )
                    o_sb = po.tile([128, CH], bf16, name="osb", tag="o")
                    nc.vector.tensor_copy(out=o_sb, in_=ps)
                    nsp = 2
                    w = CH // nsp
                    for e in range(nsp):
                        nc.sync.dma_start(
                            out=outT[
                                128 * nt:128 * (nt + 1),
                                CH * j + w * e:CH * j + w * (e + 1),
                            ],
                            in_=o_sb[:, w * e:w * (e + 1)],
                        )
                yield og

        # ---------- chunk 0 QKV up front ----------
        # pad the stream with DMA-independent warm matmuls (into the
        # not-yet-used scores PSUM pool) so input-DMA jitter neither
        # stalls the in-order PE nor resets its p-state ramp
        for i, unit in enumerate(qkv_units(0)):
            unit()
            if i < 12:
                ps = pp_sc.tile([128, 2 * CH], f32, name="sc", tag="sc")
                nc.tensor.matmul(
                    ps[:, 0:CH], lhsT=warm[:, 0:128], rhs=warm,
                    start=True, stop=True,
                )

        # ---------- main loop: attention(j) with interleaved fillers ----------
        for j in range(NCH):
            fillers = []
            if j + 1 < NCH:
                fillers.extend(qkv_units(j + 1))
            if j == NCH - 1:
                for jo in range(NCH - 1):
                    fillers.extend(outproj_units(jo))
            nkt = 4 * (j + 1)
            # at j=3, under-pop so a few og fillers bridge the final
            # normalize chain before the last out-projection
            n_units = NMT * (nkt + 2) + (4 if j == NCH - 1 else 0)
            n_fill = len(fillers)
            popped = 0
            ucount = 0

            qt = qt_all[j]
            ct = []
            ct_all[j] = ct
            for t in range(NMT):
                av = [
                    pp_av.tile([128, CH], f32, name=f"av{h}", tag="av")
                    for h in range(2)
                ]
                us = {}
                for kt in range(nkt + 2):
                    if kt < nkt:
                        dd = kt - 4 * j      # diagonal index (>=0 on diag)
                        qoff = 128 * dd if dd >= 0 else 0
                        n = CH - qoff
                        ck, ks = kt // 4, (kt % 4) * 128
                        # both heads' scores in one 2-bank PSUM tile
                        sc = pp_sc.tile([128, 2 * CH], f32, name="sc", tag="sc")
                        for h in range(2):
                            pb = 64 * h
                            nc.tensor.matmul(
                                sc[:, CH * h:CH * h + n],
                                lhsT=kt_sb[t][ck][pb:pb + 64, ks:ks + 128],
                                rhs=qt[t][pb:pb + 64, qoff:CH],
                                start=True,
                                stop=True,
                                tile_position=(pb, 0),
                            )
                        u = pu.tile([128, 2 * CH], bf16, name="u", tag="u")
                        scv = sc.rearrange("p (h q) -> p h q", h=2)[:, :, 0:n]
                        uv = u.rearrange("p (h q) -> p h q", h=2)[:, :, 0:n]
                        nc.scalar.activation(out=uv, in_=scv, func=EXP, scale=0.125)
                        if dd >= 0:
                            # keep where q_rel >= k_partition (same mask, both)
                            nc.gpsimd.affine_select(
                                out=uv,
                                in_=uv,
                                compare_op=mybir.AluOpType.is_ge,
                                fill=0.0,
                                base=0,
                                channel_multiplier=-1,
                                pattern=[[0, 2], [1, n]],
                            )
                        us[kt] = (u, qoff, n)
                    # fillers split around the attnV pair: the in-order PE
                    # chews on independent work while ScalarE finishes exp,
                    # and single-buffered mm-PSUM drains overlap attention
                    ucount += 1
                    if fillers and popped < ucount * n_fill // n_units:
                        fillers.pop(0)()
                        popped += 1
                    if kt >= 2:
                        # attnV runs TWO k-tiles behind scores so exp (which
                        # takes ~1.5 score-pairs of ScalarE time) never
                        # stalls the in-order PE stream
                        pkt = kt - 2
                        u_p, qoff_p, n_p = us.pop(pkt)
                        for h in range(2):
                            ha = 2 * t + h
                            nc.tensor.matmul(
                                av[h][:, qoff_p:CH],
                                lhsT=vau[pkt][:, 128 * ha:128 * ha + 128],
                                rhs=u_p[:, CH * h:CH * h + n_p],
                                start=(pkt == 0),
                                stop=(pkt == nkt - 1),
                            )
                    while fillers and popped < ucount * n_fill // n_units:
                        fillers.pop(0)()
                        popped += 1

                # normalize: denominator is replicated on PSUM partitions
                # 64-127 by the ones-columns; partition-aligned recip + mul
                c_t = pct.tile([128, CH], bf16, name=f"c{t}_{j}", tag=f"c{t}")
                for h in range(2):
                    # approx-recip's bitwise seed needs an SBUF fp32 input;
                    # stage the replicated denominator rows out of PSUM first.
                    # ct isn't consumed until the last chunk, so for j<3 the
                    # stage can ride on ScalarE's slack instead of DVE.
                    den = prc.tile([64, CH], f32, name=f"den{h}", tag="den")
                    if j < 3:
                        nc.scalar.copy(out=den, in_=av[h][64:128, :])
                    else:
                        nc.vector.tensor_copy(out=den, in_=av[h][64:128, :])
                    rec = prc.tile([64, CH], f32, name=f"rec{h}", tag="rec")
                    nc.vector.reciprocal_approx_fast(out=rec, in_=den)
                    nc.vector.tensor_mul(
                        c_t[64 * h:64 * (h + 1), :], av[h][0:64, :], rec
                    )
                ct.append(c_t)

            # leftover fillers for this round
            for f in fillers:
                f()

        # final chunk's out-projection
        for unit in outproj_units(NCH - 1):
            unit()


_PROG = None


def _build():
    global _PROG
    if _PROG is not None:
        return _PROG
    import concourse.bacc as bacc
    import concourse.mybir as mybir
    import concourse.tile as tile

    bf16 = mybir.dt.bfloat16
    nc = bacc.Bacc(
        "TRN2", target_bir_lowering=False, debug=False, enable_asserts=False
    )
    xT = nc.dram_tensor("xT", [128, NCH * ND * CH], bf16, kind="ExternalInput").ap()
    wq = nc.dram_tensor("wq", [128, ND * M], bf16, kind="ExternalInput").ap()
    wk = nc.dram_tensor("wk", [128, ND * M], bf16, kind="ExternalInput").ap()
    wv = nc.dram_tensor("wv", [128, ND * M], bf16, kind="ExternalInput").ap()
    wo = nc.dram_tensor("wo", [128, NMT * D], bf16, kind="ExternalInput").ap()
    outT = nc.dram_tensor("outT", [D, S], bf16, kind="ExternalOutput").ap()

    with tile.TileContext(nc) as tc:
        _emit(nc, tc, tile, mybir, (xT, wq, wk, wv, wo, outT))
    nc.compile()
    _PROG = nc
    return nc


def kernel(x, Wq, Wk, Wv, Wo, bo):
    global LAST_RESULT
    import os

    from concourse.bass_utils import run_bass_kernel_spmd

    x = np.asarray(x, dtype=np.float32)
    Wq = np.asarray(Wq, dtype=np.float32)
    Wk = np.asarray(Wk, dtype=np.float32)
    Wv = np.asarray(Wv, dtype=np.float32)
    Wo = np.asarray(Wo, dtype=np.float32)
    bo = np.asarray(bo, dtype=np.float32)

    nc = _build()

    import ml_dtypes

    bf = ml_dtypes.bfloat16

    def fold_w(w):
        # [(nd p), c] -> [p, (nd c)]
        ndt = w.shape[0] // 128
        return np.ascontiguousarray(
            w.reshape(ndt, 128, w.shape[1]).transpose(1, 0, 2).reshape(128, -1)
        ).astype(bf)

    in_maps = []
    for c in range(NCORE):
        b, g = c // 2, c % 2
        cols = slice(M * g, M * (g + 1))
        xt = x[b].T  # [D, S]
        # [p, (j d s)]: xf[p, j*ND*CH + d*CH + s] = xT[128d+p, CH*j+s]
        xf = (
            xt.reshape(ND, 128, NCH, CH)
            .transpose(1, 2, 0, 3)
            .reshape(128, NCH * ND * CH)
        )
        in_maps.append(
            {
                "xT": np.ascontiguousarray(xf).astype(bf),
                "wq": fold_w(Wq[:, cols]),
                "wk": fold_w(Wk[:, cols]),
                "wv": fold_w(Wv[:, cols]),
                "wo": fold_w(Wo[cols, :]),
            }
        )

    res = run_bass_kernel_spmd(
        nc,
        in_maps,
        list(range(NCORE)),
        trace=bool(os.environ.get("KERNEL_TRACE")),
        tmpdir=os.environ.get("KERNEL_TRACE_DIR") or None,
    )
    LAST_RESULT = res

    out = np.empty((B, S, D), dtype=np.float32)
    for b in range(B):
        acc = res.results[2 * b]["outT"].astype(np.float32) + res.results[
            2 * b + 1
        ]["outT"].astype(np.float32)
        out[b] = acc.T + bo[None, :]
    return out
